# revision 14
# baseline (speedup 1.0000x reference)
"""Trainium2 Bass kernel for nn_Attention_linearCombination.

out = sum_i softmax_i(tanh(x_i @ W_att_i + b_att_i) @ v) * (x_i @ W_tr_i + b_tr_i)

Sharding: data-parallel over the batch dim (16384 -> 8 cores x 2048 rows);
weights replicated.

v2 design (from TimelineSim analysis of the v1 kernel: PE starved on
stationary reloads at mid p-state, ACT doing 6 full passes/tile, DMA
reading fp32 from DRAM):
  - x and all weights are cast to bf16 ON HOST and declared as bf16 DRAM
    tensors: halves HBM read traffic (the old kernel cast during the DMA,
    paying fp32 reads).
  - loads/transposes batched in 2-tile pairs (fewer SWDGE/HWDGE issues);
    loads 2 pairs ahead on SWDGE q0, transposes 1 pair ahead on the SP
    HWDGE ring, stores on SWDGE q1 so they never block loads.
  - per (tile, branch): ONE accumulation pass; group starts with K=1
    bias matmuls (ones stationary shared by b_att and b_tr), then per
    k-chunk the stationary xT chunk is loaded ONCE and shared by the att
    (N=256) and tr (N=512) matmuls (interleaved PSUM groups).  Stationary
    loads drop ~2x vs v1; PE stream is continuous for full p-state.
  - tanh via sigmoid identity (tanh(a) = 2*sigmoid(2a) - 1; Tanh ACT table
    crashes the device on this runtime); since b_att is inside the PSUM
    group, sgh = sigmoid(2*p_att) straight from PSUM.
  - softmax via sigma-ratio (e^l = sig(l)/(1-sig(l))) to stay on the
    sigmoid ACT table set (exp lives in a different set -> reload thrash;
    tensor_tensor_reduce also crashes the device -> separate mul+reduce).
  - combine: SCALED evacuation (ACT Copy with per-partition scale s3[:,i])
    directly PSUM->SBUF bf16, then two DVE adds (first in bf16 2x mode).
"""
import numpy as np
import ml_dtypes

import concourse.bass as bass
import concourse.bacc as bacc
import concourse.mybir as mybir
import concourse.tile as tile
from concourse.bass_utils import run_bass_kernel_spmd

F32 = mybir.dt.float32
BF16 = mybir.dt.bfloat16
AF = mybir.ActivationFunctionType
OP = mybir.AluOpType

B = 16384
D = 1024
INT = 256
OUT = 512
NB = 3
NCORES = 8
B_LOC = B // NCORES
KC = D // 128
N_TILES = B_LOC // 128
PAIR = 2  # tiles per load/transpose batch
N_PAIRS = N_TILES // PAIR

INTERLEAVE = True  # share each stationary xT chunk between att+tr matmuls

_CACHE = {}


def _build_nc(repeat=1, loop_repeat=1, interleave=None):
    if interleave is None:
        interleave = INTERLEAVE
    nc = bacc.Bacc(None, target_bir_lowering=False, num_swdge_queues=2)
    xs = [nc.dram_tensor(f"x{i+1}", [B_LOC, D], BF16, kind="ExternalInput") for i in range(NB)]
    Was = [nc.dram_tensor(f"W_att{i+1}", [D, INT], BF16, kind="ExternalInput") for i in range(NB)]
    bas = [nc.dram_tensor(f"b_att{i+1}", [1, INT], BF16, kind="ExternalInput") for i in range(NB)]
    Wts = [nc.dram_tensor(f"W_tr{i+1}", [D, OUT], BF16, kind="ExternalInput") for i in range(NB)]
    bts = [nc.dram_tensor(f"b_tr{i+1}", [1, OUT], BF16, kind="ExternalInput") for i in range(NB)]
    v = nc.dram_tensor("v", [INT, 1], F32, kind="ExternalInput")
    out = nc.dram_tensor("out", [B_LOC, OUT], F32, kind="ExternalOutput")

    with tile.TileContext(nc) as tc:
        with (
            tc.tile_pool(name="wpool", bufs=1) as wpool,
            tc.tile_pool(name="work", bufs=4) as pool,
            tc.tile_pool(name="xbpool", bufs=4) as xbpool,
            tc.tile_pool(name="xpool", bufs=3) as xpool,
            tc.tile_pool(name="tpool", bufs=4) as tpool,
            tc.tile_pool(name="psum", bufs=2, space="PSUM") as psum,
            tc.tile_pool(name="ptr", bufs=6, space="PSUM") as ptrpool,
        ):
            # ---- one-time setup ----
            # biases on the ACT HWDGE ring (tiny transfers, nothing queued
            # behind them); weight matrices interleaved with the first x
            # loads on the Pool ring below so each branch's W lands just
            # before its first use.
            Wa_sb, Wt_sb, ba_sb, bt_sb = [], [], [], []
            for i in range(NB):
                wa = wpool.tile([128, KC, INT], BF16, tag=f"wa{i}")
                Wa_sb.append(wa)
                wt = wpool.tile([128, KC, OUT], BF16, tag=f"wt{i}")
                Wt_sb.append(wt)
                bav = wpool.tile([1, INT], BF16, tag=f"ba{i}")
                nc.scalar.dma_start(out=bav[:], in_=bas[i][:])
                ba_sb.append(bav)
                btv = wpool.tile([1, OUT], BF16, tag=f"bt{i}")
                nc.scalar.dma_start(out=btv[:], in_=bts[i][:])
                bt_sb.append(btv)
            ones16 = wpool.tile([1, 128], BF16, tag="ones16")
            nc.vector.memset(ones16[:], 1.0)
            ones32 = wpool.tile([1, 128], F32, tag="ones32")
            nc.vector.memset(ones32[:], 1.0)

            # v broadcast to all partitions via K=1 fp32 matmul
            v_row = wpool.tile([1, INT], F32, tag="vrow")
            nc.sync.dma_start(out=v_row[:], in_=v.rearrange("a b -> b a"))
            p_v0 = psum.tile([128, 2, INT], F32, tag="att2")
            p_v = p_v0[:, 0, :]
            nc.tensor.matmul(p_v, lhsT=ones32[:], rhs=v_row[:], start=True, stop=True)
            v2_rep = wpool.tile([128, INT], F32, tag="v2rep")
            nc.scalar.activation(v2_rep[:], p_v, AF.Copy, scale=2.0)
            vsum = wpool.tile([128, 1], F32, tag="vsum")
            nc.vector.reduce_sum(vsum[:], p_v, axis=mybir.AxisListType.X)

            # ---- main loop over tile-pairs, software-pipelined ----
            import contextlib
            loop_cm = tc.For_i(0, loop_repeat, 1) if loop_repeat > 1 else contextlib.nullcontext()
            with loop_cm:
              xb_q, xT_q = {}, {}

              def issue_loads(p):
                  for i in range(NB):
                      xb = xbpool.tile([128, PAIR, D], BF16, tag=f"xb{i}")
                      nc.gpsimd.dma_start(
                          out=xb[:],
                          in_=xs[i][p * 128 * PAIR:(p + 1) * 128 * PAIR, :]
                              .rearrange("(a p) d -> p a d", p=128))
                      xb_q[(p, i)] = xb

              def issue_transposes(p):
                  for i in range(NB):
                      xT = xpool.tile([128, PAIR * KC, 128], BF16, tag=f"xT{i}")
                      nc.sync.dma_start(out=xT[:], in_=xb_q.pop((p, i))[:], transpose=True)
                      xT_q[(p, i)] = xT

              def load_W(i):
                  nc.scalar.dma_start(out=Wa_sb[i][:],
                                      in_=Was[i].rearrange("(c p) n -> p c n", p=128))
                  nc.scalar.dma_start(out=Wt_sb[i][:],
                                      in_=Wts[i].rearrange("(c p) n -> p c n", p=128))

              pairs = [pp for _ in range(repeat) for pp in range(N_PAIRS)]
              load_W(0)
              issue_loads(pairs[0])
              load_W(1)
              issue_loads(pairs[1])
              load_W(2)
              issue_loads(pairs[2])
              issue_transposes(pairs[0])
              for pi, p in enumerate(pairs):
                if pi + 3 < len(pairs):
                    issue_loads(pairs[pi + 3])
                if pi + 1 < len(pairs):
                    issue_transposes(pairs[pi + 1])
                xTs = [xT_q.pop((p, i)) for i in range(NB)]
                for t2 in range(PAIR):
                    t = p * PAIR + t2
                    l3 = pool.tile([128, 4], F32, tag="l3")
                    pa01 = psum.tile([128, 2, INT], F32, tag="att2")
                    pa2 = psum.tile([128, 2, INT], F32, tag="att2")
                    att_slots = [pa01[:, 0, :], pa01[:, 1, :], pa2[:, 0, :]]
                    p_atts, p_trs = [], []
                    for i in range(NB):
                        p_att = att_slots[i]
                        p_tr = ptrpool.tile([128, OUT], F32, tag="tr")
                        # bias first: one ones-stationary load serves both
                        nc.tensor.matmul(p_att, lhsT=ones16[:], rhs=ba_sb[i][:],
                                         start=True, stop=False)
                        nc.tensor.matmul(p_tr[:], lhsT=ones16[:], rhs=bt_sb[i][:],
                                         start=True, stop=False)
                        if interleave:
                            for c in range(KC):
                                xc = xTs[i][:, t2 * KC + c, :]
                                nc.tensor.matmul(p_att, lhsT=xc, rhs=Wa_sb[i][:, c, :],
                                                 start=False, stop=(c == KC - 1))
                                nc.tensor.matmul(p_tr[:], lhsT=xc, rhs=Wt_sb[i][:, c, :],
                                                 start=False, stop=(c == KC - 1))
                        else:
                            for c in range(KC):
                                nc.tensor.matmul(p_att, lhsT=xTs[i][:, t2 * KC + c, :],
                                                 rhs=Wa_sb[i][:, c, :],
                                                 start=False, stop=(c == KC - 1))
                            for c in range(KC):
                                nc.tensor.matmul(p_tr[:], lhsT=xTs[i][:, t2 * KC + c, :],
                                                 rhs=Wt_sb[i][:, c, :],
                                                 start=False, stop=(c == KC - 1))
                        p_atts.append(p_att)
                        p_trs.append(p_tr)

                    for i in range(NB):
                        # tanh identity: l_i = sgh @ (2v) - sum(v), sgh = sig(2(xWa+ba))
                        sgh = pool.tile([128, INT], F32, tag="sgh")
                        nc.scalar.activation(sgh[:], p_atts[i], AF.Sigmoid, scale=2.0)
                        prod = pool.tile([128, INT], F32, tag="prod")
                        nc.vector.tensor_mul(prod[:], sgh[:], v2_rep[:])
                        raw = pool.tile([128, 1], F32, tag="raw")
                        nc.vector.reduce_sum(raw[:], prod[:], axis=mybir.AxisListType.X)
                        nc.vector.tensor_scalar(l3[:, i:i + 1], raw[:], vsum[:], None, OP.subtract)

                    # softmax over 3 branches via sigma-ratio, all on [128, 4] tiles
                    sg3 = pool.tile([128, 4], F32, tag="sg3")
                    nc.scalar.activation(sg3[:, 0:NB], l3[:, 0:NB], AF.Sigmoid)
                    u3 = pool.tile([128, 4], F32, tag="u3")
                    nc.vector.tensor_scalar(u3[:, 0:NB], sg3[:, 0:NB], -1.0, 1.0, OP.mult, OP.add)
                    w3 = pool.tile([128, 4], F32, tag="w3")
                    nc.vector.reciprocal(w3[:, 0:NB], u3[:, 0:NB])
                    r3 = pool.tile([128, 4], F32, tag="r3")
                    nc.vector.tensor_mul(r3[:, 0:NB], sg3[:, 0:NB], w3[:, 0:NB])
                    ssum = pool.tile([128, 1], F32, tag="ssum")
                    nc.vector.reduce_sum(ssum[:], r3[:, 0:NB], axis=mybir.AxisListType.X)
                    rs = pool.tile([128, 1], F32, tag="rs")
                    nc.vector.reciprocal(rs[:], ssum[:])
                    s3 = pool.tile([128, 4], F32, tag="s3")
                    nc.vector.tensor_scalar_mul(s3[:, 0:NB], r3[:, 0:NB], rs[:])

                    # scaled evacuation + two adds, all on DVE: the s3
                    # dependency stays DVE-local and tr banks free here
                    t0 = tpool.tile([128, OUT], BF16, tag="t0")
                    t1 = tpool.tile([128, OUT], BF16, tag="t1")
                    t2s = tpool.tile([128, OUT], BF16, tag="t2")
                    for i, tt in enumerate([t0, t1, t2s]):
                        nc.vector.tensor_scalar_mul(tt[:], p_trs[i][:], s3[:, i:i + 1])
                    a01 = tpool.tile([128, OUT], BF16, tag="a01")
                    nc.vector.tensor_add(a01[:], t0[:], t1[:])
                    acc = tpool.tile([128, OUT], F32, tag="acc")
                    nc.vector.tensor_add(acc[:], a01[:], t2s[:])
                    # store on the Pool SWDGE ring behind the (3-pair-ahead)
                    # loads: by the time a store reaches the queue head its
                    # acc is ~ready, so it never starves the load stream
                    nc.gpsimd.dma_start(out=out[t * 128:(t + 1) * 128, :], in_=acc[:])
    nc.compile()
    return nc


def make_in_maps(inputs):
    """Shard + cast FULL fp32 inputs into per-core in_maps (x/W/b in bf16)."""
    bf = ml_dtypes.bfloat16
    shared = {}
    for i in range(NB):
        for k in (f"W_att{i+1}", f"b_att{i+1}", f"W_tr{i+1}", f"b_tr{i+1}"):
            shared[k] = np.ascontiguousarray(np.asarray(inputs[k]).astype(bf))
    shared["v"] = np.ascontiguousarray(np.asarray(inputs["v"], dtype=np.float32))
    in_maps = []
    for c in range(NCORES):
        m = dict(shared)
        for i in range(NB):
            m[f"x{i+1}"] = np.ascontiguousarray(
                np.asarray(inputs[f"x{i+1}"])[c * B_LOC:(c + 1) * B_LOC].astype(bf)
            )
        in_maps.append(m)
    return in_maps


LAST_RESULTS = None


def kernel(**inputs) -> np.ndarray:
    if "nc" not in _CACHE:
        _CACHE["nc"] = _build_nc()
    nc = _CACHE["nc"]

    in_maps = make_in_maps(inputs)
    res = run_bass_kernel_spmd(nc, in_maps, core_ids=list(range(NCORES)))
    global LAST_RESULTS
    LAST_RESULTS = res
    return np.concatenate([r["out"] for r in res.results], axis=0)


# revision 17
# speedup vs baseline: 10.7380x; 10.7380x over previous
"""Trainium2 Bass kernel for nn_Attention_linearCombination.

out = sum_i softmax_i(tanh(x_i @ W_att_i + b_att_i) @ v) * (x_i @ W_tr_i + b_tr_i)

Sharding: data-parallel over the batch dim (16384 -> 8 cores x 2048 rows);
weights replicated.

v2 design (from TimelineSim analysis of the v1 kernel: PE starved on
stationary reloads at mid p-state, ACT doing 6 full passes/tile, DMA
reading fp32 from DRAM):
  - x and all weights are cast to bf16 ON HOST and declared as bf16 DRAM
    tensors: halves HBM read traffic (the old kernel cast during the DMA,
    paying fp32 reads).
  - loads/transposes batched in 2-tile pairs (fewer SWDGE/HWDGE issues);
    loads 2 pairs ahead on SWDGE q0, transposes 1 pair ahead on the SP
    HWDGE ring, stores on SWDGE q1 so they never block loads.
  - per (tile, branch): ONE accumulation pass; group starts with K=1
    bias matmuls (ones stationary shared by b_att and b_tr), then per
    k-chunk the stationary xT chunk is loaded ONCE and shared by the att
    (N=256) and tr (N=512) matmuls (interleaved PSUM groups).  Stationary
    loads drop ~2x vs v1; PE stream is continuous for full p-state.
  - tanh via sigmoid identity (tanh(a) = 2*sigmoid(2a) - 1; Tanh ACT table
    crashes the device on this runtime); since b_att is inside the PSUM
    group, sgh = sigmoid(2*p_att) straight from PSUM.
  - softmax via sigma-ratio (e^l = sig(l)/(1-sig(l))) to stay on the
    sigmoid ACT table set (exp lives in a different set -> reload thrash;
    tensor_tensor_reduce also crashes the device -> separate mul+reduce).
  - combine: SCALED evacuation (ACT Copy with per-partition scale s3[:,i])
    directly PSUM->SBUF bf16, then two DVE adds (first in bf16 2x mode).
"""
import numpy as np
import ml_dtypes

import concourse.bass as bass
import concourse.bacc as bacc
import concourse.mybir as mybir
import concourse.tile as tile
from concourse.bass_utils import run_bass_kernel_spmd

F32 = mybir.dt.float32
BF16 = mybir.dt.bfloat16
AF = mybir.ActivationFunctionType
OP = mybir.AluOpType

B = 16384
D = 1024
INT = 256
OUT = 512
NB = 3
NCORES = 8
B_LOC = B // NCORES
KC = D // 128
N_TILES = B_LOC // 128
PAIR = 2  # tiles per load/transpose batch
N_PAIRS = N_TILES // PAIR

INTERLEAVE = True  # share each stationary xT chunk between att+tr matmuls

_CACHE = {}


def _build_nc(repeat=1, loop_repeat=1, interleave=None, mode="full", direct_t=False):
    if interleave is None:
        interleave = INTERLEAVE
    nc = bacc.Bacc(None, target_bir_lowering=False, num_swdge_queues=2)
    xs = [nc.dram_tensor(f"x{i+1}", [B_LOC, D], BF16, kind="ExternalInput") for i in range(NB)]
    Was = [nc.dram_tensor(f"W_att{i+1}", [D, INT], BF16, kind="ExternalInput") for i in range(NB)]
    bas = [nc.dram_tensor(f"b_att{i+1}", [1, INT], BF16, kind="ExternalInput") for i in range(NB)]
    Wts = [nc.dram_tensor(f"W_tr{i+1}", [D, OUT], BF16, kind="ExternalInput") for i in range(NB)]
    bts = [nc.dram_tensor(f"b_tr{i+1}", [1, OUT], BF16, kind="ExternalInput") for i in range(NB)]
    v = nc.dram_tensor("v", [INT, 1], F32, kind="ExternalInput")
    out = nc.dram_tensor("out", [B_LOC, OUT], F32, kind="ExternalOutput")

    with tile.TileContext(nc) as tc:
        with (
            tc.tile_pool(name="wpool", bufs=1) as wpool,
            tc.tile_pool(name="work", bufs=4) as pool,
            tc.tile_pool(name="xbpool", bufs=4) as xbpool,
            tc.tile_pool(name="xpool", bufs=3) as xpool,
            tc.tile_pool(name="tpool", bufs=4) as tpool,
            tc.tile_pool(name="psum", bufs=2, space="PSUM") as psum,
            tc.tile_pool(name="ptr", bufs=6, space="PSUM") as ptrpool,
        ):
            # ---- one-time setup ----
            # biases on the ACT HWDGE ring (tiny transfers, nothing queued
            # behind them); weight matrices interleaved with the first x
            # loads on the Pool ring below so each branch's W lands just
            # before its first use.
            Wa_sb, Wt_sb, ba_sb, bt_sb = [], [], [], []
            for i in range(NB):
                wa = wpool.tile([128, KC, INT], BF16, tag=f"wa{i}")
                Wa_sb.append(wa)
                wt = wpool.tile([128, KC, OUT], BF16, tag=f"wt{i}")
                Wt_sb.append(wt)
                bav = wpool.tile([1, INT], BF16, tag=f"ba{i}")
                nc.scalar.dma_start(out=bav[:], in_=bas[i][:])
                ba_sb.append(bav)
                btv = wpool.tile([1, OUT], BF16, tag=f"bt{i}")
                nc.scalar.dma_start(out=btv[:], in_=bts[i][:])
                bt_sb.append(btv)
            ones16 = wpool.tile([1, 128], BF16, tag="ones16")
            nc.vector.memset(ones16[:], 1.0)
            ones32 = wpool.tile([1, 128], F32, tag="ones32")
            nc.vector.memset(ones32[:], 1.0)

            # v broadcast to all partitions via K=1 fp32 matmul
            v_row = wpool.tile([1, INT], F32, tag="vrow")
            nc.sync.dma_start(out=v_row[:], in_=v.rearrange("a b -> b a"))
            p_v0 = psum.tile([128, 2, INT], F32, tag="att2")
            p_v = p_v0[:, 0, :]
            nc.tensor.matmul(p_v, lhsT=ones32[:], rhs=v_row[:], start=True, stop=True)
            v2_rep = wpool.tile([128, INT], F32, tag="v2rep")
            nc.scalar.activation(v2_rep[:], p_v, AF.Copy, scale=2.0)
            vsum = wpool.tile([128, 1], F32, tag="vsum")
            nc.vector.reduce_sum(vsum[:], p_v, axis=mybir.AxisListType.X)

            # ---- main loop over tile-pairs, software-pipelined ----
            import contextlib
            loop_cm = tc.For_i(0, loop_repeat, 1) if loop_repeat > 1 else contextlib.nullcontext()
            with loop_cm:
              xb_q, xT_q = {}, {}

              def issue_loads(p):
                  if direct_t:
                      return
                  for i in range(NB):
                      xb = xbpool.tile([128, PAIR, D], BF16, tag=f"xb{i}")
                      nc.gpsimd.dma_start(
                          out=xb[:],
                          in_=xs[i][p * 128 * PAIR:(p + 1) * 128 * PAIR, :]
                              .rearrange("(a p) d -> p a d", p=128))
                      xb_q[(p, i)] = xb

              def issue_transposes(p):
                  for i in range(NB):
                      if direct_t:
                          # xbar transpose straight from DRAM (x is bf16):
                          # out[p, c, j] = x[row j, d = c*128 + p]
                          xT = xpool.tile([128, KC, PAIR * 128], BF16, tag=f"xT{i}")
                          nc.sync.dma_start(
                              out=xT[:],
                              in_=xs[i][p * 128 * PAIR:(p + 1) * 128 * PAIR, :],
                              transpose=True)
                      else:
                          xT = xpool.tile([128, PAIR * KC, 128], BF16, tag=f"xT{i}")
                          nc.sync.dma_start(out=xT[:], in_=xb_q.pop((p, i))[:], transpose=True)
                      xT_q[(p, i)] = xT

              def xc_slice(xT, t2, c):
                  if direct_t:
                      return xT[:, c, t2 * 128:(t2 + 1) * 128]
                  return xT[:, t2 * KC + c, :]

              def load_W(i):
                  nc.scalar.dma_start(out=Wa_sb[i][:],
                                      in_=Was[i].rearrange("(c p) n -> p c n", p=128))
                  nc.scalar.dma_start(out=Wt_sb[i][:],
                                      in_=Wts[i].rearrange("(c p) n -> p c n", p=128))

              pairs = [pp for _ in range(repeat) for pp in range(N_PAIRS)]
              load_W(0)
              issue_loads(pairs[0])
              load_W(1)
              issue_loads(pairs[1])
              load_W(2)
              issue_loads(pairs[2])
              issue_transposes(pairs[0])
              for pi, p in enumerate(pairs):
                if pi + 3 < len(pairs):
                    issue_loads(pairs[pi + 3])
                if pi + 1 < len(pairs):
                    issue_transposes(pairs[pi + 1])
                xTs = [xT_q.pop((p, i)) for i in range(NB)]
                for t2 in range(PAIR):
                    t = p * PAIR + t2
                    pa01 = psum.tile([128, 2, INT], F32, tag="att2")
                    pa2 = psum.tile([128, 2, INT], F32, tag="att2")
                    att_slots = [pa01[:, 0, :], pa01[:, 1, :], pa2[:, 0, :]]
                    p_atts, p_trs = [], []
                    for i in range(NB):
                        p_att = att_slots[i]
                        p_tr = ptrpool.tile([128, OUT], F32, tag="tr")
                        # bias first: one ones-stationary load serves both
                        nc.tensor.matmul(p_att, lhsT=ones16[:], rhs=ba_sb[i][:],
                                         start=True, stop=False)
                        nc.tensor.matmul(p_tr[:], lhsT=ones16[:], rhs=bt_sb[i][:],
                                         start=True, stop=False)
                        if interleave:
                            for c in range(KC):
                                xc = xc_slice(xTs[i], t2, c)
                                nc.tensor.matmul(p_att, lhsT=xc, rhs=Wa_sb[i][:, c, :],
                                                 start=False, stop=(c == KC - 1))
                                nc.tensor.matmul(p_tr[:], lhsT=xc, rhs=Wt_sb[i][:, c, :],
                                                 start=False, stop=(c == KC - 1))
                        else:
                            for c in range(KC):
                                nc.tensor.matmul(p_att, lhsT=xc_slice(xTs[i], t2, c),
                                                 rhs=Wa_sb[i][:, c, :],
                                                 start=False, stop=(c == KC - 1))
                            for c in range(KC):
                                nc.tensor.matmul(p_tr[:], lhsT=xc_slice(xTs[i], t2, c),
                                                 rhs=Wt_sb[i][:, c, :],
                                                 start=False, stop=(c == KC - 1))
                        p_atts.append(p_att)
                        p_trs.append(p_tr)

                    if mode == "pe":
                        continue
                    l3 = pool.tile([128, 4], F32, tag="l3")
                    for i in range(NB):
                        # tanh identity: l_i = sgh @ (2v) - sum(v), sgh = sig(2(xWa+ba))
                        sgh = pool.tile([128, INT], F32, tag="sgh")
                        nc.scalar.activation(sgh[:], p_atts[i], AF.Sigmoid, scale=2.0)
                        prod = pool.tile([128, INT], F32, tag="prod")
                        nc.vector.tensor_mul(prod[:], sgh[:], v2_rep[:])
                        raw = pool.tile([128, 1], F32, tag="raw")
                        nc.vector.reduce_sum(raw[:], prod[:], axis=mybir.AxisListType.X)
                        nc.vector.tensor_scalar(l3[:, i:i + 1], raw[:], vsum[:], None, OP.subtract)

                    # softmax over 3 branches via sigma-ratio, all on [128, 4] tiles
                    sg3 = pool.tile([128, 4], F32, tag="sg3")
                    nc.scalar.activation(sg3[:, 0:NB], l3[:, 0:NB], AF.Sigmoid)
                    u3 = pool.tile([128, 4], F32, tag="u3")
                    nc.vector.tensor_scalar(u3[:, 0:NB], sg3[:, 0:NB], -1.0, 1.0, OP.mult, OP.add)
                    w3 = pool.tile([128, 4], F32, tag="w3")
                    nc.vector.reciprocal(w3[:, 0:NB], u3[:, 0:NB])
                    r3 = pool.tile([128, 4], F32, tag="r3")
                    nc.vector.tensor_mul(r3[:, 0:NB], sg3[:, 0:NB], w3[:, 0:NB])
                    ssum = pool.tile([128, 1], F32, tag="ssum")
                    nc.vector.reduce_sum(ssum[:], r3[:, 0:NB], axis=mybir.AxisListType.X)
                    rs = pool.tile([128, 1], F32, tag="rs")
                    nc.vector.reciprocal(rs[:], ssum[:])
                    s3 = pool.tile([128, 4], F32, tag="s3")
                    nc.vector.tensor_scalar_mul(s3[:, 0:NB], r3[:, 0:NB], rs[:])

                    # scaled evacuation + two adds, all on DVE: the s3
                    # dependency stays DVE-local and tr banks free here
                    t0 = tpool.tile([128, OUT], BF16, tag="t0")
                    t1 = tpool.tile([128, OUT], BF16, tag="t1")
                    t2s = tpool.tile([128, OUT], BF16, tag="t2")
                    for i, tt in enumerate([t0, t1, t2s]):
                        nc.vector.tensor_scalar_mul(tt[:], p_trs[i][:], s3[:, i:i + 1])
                    a01 = tpool.tile([128, OUT], BF16, tag="a01")
                    nc.vector.tensor_add(a01[:], t0[:], t1[:])
                    acc = tpool.tile([128, OUT], F32, tag="acc")
                    nc.vector.tensor_add(acc[:], a01[:], t2s[:])
                    # store on the Pool SWDGE ring behind the (3-pair-ahead)
                    # loads: by the time a store reaches the queue head its
                    # acc is ~ready, so it never starves the load stream
                    nc.gpsimd.dma_start(out=out[t * 128:(t + 1) * 128, :], in_=acc[:])
    nc.compile()
    return nc


def make_in_maps(inputs):
    """Shard + cast FULL fp32 inputs into per-core in_maps (x/W/b in bf16)."""
    bf = ml_dtypes.bfloat16
    shared = {}
    for i in range(NB):
        for k in (f"W_att{i+1}", f"b_att{i+1}", f"W_tr{i+1}", f"b_tr{i+1}"):
            shared[k] = np.ascontiguousarray(np.asarray(inputs[k]).astype(bf))
    shared["v"] = np.ascontiguousarray(np.asarray(inputs["v"], dtype=np.float32))
    in_maps = []
    for c in range(NCORES):
        m = dict(shared)
        for i in range(NB):
            m[f"x{i+1}"] = np.ascontiguousarray(
                np.asarray(inputs[f"x{i+1}"])[c * B_LOC:(c + 1) * B_LOC].astype(bf)
            )
        in_maps.append(m)
    return in_maps


LAST_RESULTS = None


def kernel(**inputs) -> np.ndarray:
    if "nc" not in _CACHE:
        _CACHE["nc"] = _build_nc()
    nc = _CACHE["nc"]

    in_maps = make_in_maps(inputs)
    res = run_bass_kernel_spmd(nc, in_maps, core_ids=list(range(NCORES)))
    global LAST_RESULTS
    LAST_RESULTS = res
    return np.concatenate([r["out"] for r in res.results], axis=0)
